# revision 1
# baseline (speedup 1.0000x reference)
"""Trainium2 Bass kernel for nn_AWGNIndexChannelWrapper.

Reference computation:
  rx_c = bitflip(idx_c, flip_u_c)  (9-bit symbols, per-bit XOR with (u < BER))
  rx_f = bitflip(idx_f, flip_u_f)
  out  = concat([codebook_f[rx_f].reshape(B, -1), codebook_c[rx_c].reshape(B, -1)], axis=1)

Key simplification: packing/unpacking 9-bit symbols with per-bit XOR is just
  rx = idx ^ flipmask,  flipmask = sum_k (u_k < BER) << k
and the clip is a no-op (9-bit values are already < 512).

Sharding: pure data parallel over the batch dim (64 batches -> 8 cores x 8).

v2 design: the output is written with kv_writeback instead of plain DMA.
A kv_writeback with out view [b, dhi=128, dho, n_ctx=128], ncn = n_ctx = 128,
ctx_idx = 0 writes, for batch b, src[dhi, dho, b, :] to the contiguous DRAM
run at (dhi*dho_cnt + dho)*128.  With dho_cnt = points-per-batch/128 this is
exactly the row-major [point, D] layout the reference produces, and the SWDGE
descriptor stream covers a 16-partition stripe per descriptor, so the whole
20.97MB per-core output costs ~3.6us of DMA instead of ~58us.

The gathered data must sit in SBUF as [dhi=partition, b, dho, j] (the in_ap
passed to kv_writeback is the [dhi, dho, b, j] transpose of that physical
tile; kv_writeback only uses the AP's iteration order).  Point q of batch b
lives at partition q // dho_cnt, slot q % dho_cnt -- which is precisely the
layout dma_gather produces if gather position g = c*128 + p maps to point
q = p*dho_cnt + c.  That fixes the wrapped index layout to
  W[r, col = S*8 + j2] = rx[b, q]   (S = global slot, p = 16*j2 + r)
with per-region digits (fine: dho_cnt=32): q = 512*j2 + 32*r + c.

rx is therefore computed with partition P = 16*b + r and free dims (j2, c):
the idx/flip_u loads for that layout keep >=256B descriptor runs, the
(j2,c)->(c,j2) reorder is a free on-chip AP permute fused into the i32->i16
copy, and one DRAM round trip (write [r, b, c, j2] contiguous-per-partition,
read back with a x8 zero-stride partition-group broadcast) builds W.
"""

import os

import numpy as np

import concourse.bacc as bacc
import concourse.mybir as mybir
import concourse.tile as tile
from concourse.bass_utils import run_bass_kernel_spmd

# Problem constants (hardcoded per harness contract).
BER = 0.02
BITS = 9
KC = KF = 512
B, HC, WC, HF, WF, D = 64, 32, 32, 64, 64, 128

N_CORES = 8
B_LOC = B // N_CORES          # 8 batches per core
NF = B_LOC * HF * WF          # 32768 fine points per core
NC_ = B_LOC * HC * WC         # 8192 coarse points per core
P = 128
QF = HF * WF                  # 4096 fine points per batch
QC = HC * WC                  # 1024 coarse points per batch
DHOF = QF // P                # 32 fine slots per batch (d_head_outer)
DHOC = QC // P                # 8 coarse slots per batch
FINE_ROW = QF * D             # 524288 f32 per output row (fine region)
COARSE_ROW = QC * D           # 131072 f32 per output row (coarse region)
OUT_ROW = FINE_ROW + COARSE_ROW

f32 = mybir.dt.float32
i32 = mybir.dt.int32
i16 = mybir.dt.int16

J2 = 8                        # partition-group digit (p = 16*j2 + r)
R16 = 16                      # wrapped-row digit


def _pe_prep(nc, pool):
    """One-time constants for the PE one-hot gather path."""
    f16 = mybir.dt.float16
    ident = pool.tile([P, P], f16, tag="ident")
    from concourse.masks import make_identity
    make_identity(nc, ident[:])
    ones_t = pool.tile([1, P], f16, tag="ones")
    nc.vector.memset(ones_t[:], 1.0)
    iot = pool.tile([P, 1], i32, tag="iot")
    nc.gpsimd.iota(iot[:], pattern=[[0, 1]], base=0, channel_multiplier=1)
    iotf = pool.tile([P, 1], f32, tag="iotf")
    nc.vector.tensor_copy(out=iotf[:], in_=iot[:])
    kvec = pool.tile([P, 4], f32, tag="kvec")
    for c in range(4):
        nc.vector.tensor_scalar(
            out=kvec[:, c : c + 1], in0=iotf[:], scalar1=float(128 * c),
            scalar2=None, op0=mybir.AluOpType.add,
        )
    return ident, ones_t, kvec


def _cb16(nc, pool, cb, tag):
    """Codebook as f16 chunks [128k, 4*128d] (chunk-major columns)."""
    f16 = mybir.dt.float16
    cbst = pool.tile([P, D], f32, tag="cbst")
    cbf = pool.tile([P, 4 * D], f16, tag=f"cb16{tag}")
    for c in range(4):
        nc.sync.dma_start(cbst[:], cb.ap()[128 * c : 128 * (c + 1), :])
        with nc.allow_low_precision(reason="f16 codebook, rel err 2^-11"):
            nc.vector.tensor_copy(out=cbf[:, c * D : (c + 1) * D], in_=cbst[:])
    return cbf


def _pe_region(nc, pool, pe_pool, psum, kv_tile, *, idx_dram, flip_dram,
               cbf, kvec, ones_t, ident, dho, s0, tag):
    """PE one-hot gather for slots c < s0 of every batch.

    Layout: rx computed at [p' (partition), (b, c)] where point q = dho*p'+c;
    PE-transpose gives rxrow[(b,c), p']; per slot a K=1 broadcast matmul
    replicates the slot's 128 rx values across partitions (idxb), 4 f16
    is_equal compares vs kvec build the one-hot [k, p'] chunks, and 4
    accumulating matmuls against the f16 codebook produce [p', d] in PSUM,
    copied into the kv tile.  f16 codebook keeps rel err ~2^-11 << 2e-2.
    """
    f16 = mybir.dt.float16
    NS = B_LOC * s0
    idx_flat = idx_dram.ap().rearrange("b h w -> b (h w)")
    u_flat = flip_dram.ap().rearrange("b h w k -> b (h w) k")
    idxp = pool.tile([P, NS], i32, tag=f"idxp{tag}")
    up = pool.tile([P, NS * BITS], f32, tag=f"up{tag}")
    for b in range(B_LOC):
        nc.sync.dma_start(
            idxp[:, b * s0 : (b + 1) * s0],
            idx_flat[b].rearrange("(p c) -> p c", p=P)[:, 0:s0],
        )
        nc.scalar.dma_start(
            up[:, b * s0 * BITS : (b + 1) * s0 * BITS],
            u_flat[b]
            .rearrange("(p c) k -> p c k", p=P)[:, 0:s0, :]
            .rearrange("p c k -> p (c k)"),
        )
    u_v = up[:].rearrange("p (f k) -> p f k", k=BITS)
    fmp = pool.tile([P, NS], f16, tag=f"fmp{tag}")
    tmpp = pool.tile([P, NS], f16, tag=f"tmpp{tag}")
    with nc.allow_low_precision(reason="bit sums <= 511 are exact in f16"):
        for k in range(BITS):
            dst = fmp if k == 0 else tmpp
            nc.vector.tensor_scalar(
                out=dst[:], in0=u_v[:, :, k], scalar1=BER,
                scalar2=float(1 << k), op0=mybir.AluOpType.is_lt,
                op1=mybir.AluOpType.mult,
            )
            if k:
                nc.vector.tensor_tensor(
                    out=fmp[:], in0=fmp[:], in1=tmpp[:],
                    op=mybir.AluOpType.add,
                )
    fmip = pool.tile([P, NS], i32, tag=f"fmip{tag}")
    nc.vector.tensor_copy(out=fmip[:], in_=fmp[:])
    rxp = pool.tile([P, NS], i32, tag=f"rxp{tag}")
    nc.vector.tensor_tensor(
        out=rxp[:], in0=idxp[:], in1=fmip[:], op=mybir.AluOpType.bitwise_xor
    )
    rxf = pool.tile([P, NS], f16, tag=f"rxf{tag}")
    nc.vector.tensor_copy(out=rxf[:], in_=rxp[:])

    tp = psum.tile([NS, P], f16, tag=f"tp{tag}")
    nc.tensor.transpose(tp[:], rxf[:], ident[:])
    rxrow = pool.tile([NS, P], f16, tag=f"rxrow{tag}")
    nc.vector.tensor_copy(out=rxrow[:], in_=tp[:])
    # Matmul operands must start at partition 0/32/64, so flatten each
    # 32-slot group of rxrow into a single partition-0 row via SBUF->SBUF
    # DMA; slot j's rhs is then a 128-col slice of rxG at base partition 0.
    assert NS % 32 == 0 and s0 % 4 == 0
    for j0 in range(0, NS, 4):
        g = j0 // 32
        if j0 % 32 == 0:
            rxg = pool.tile([1, 32 * P], f16, tag=f"rxg{tag}")
            nc.sync.dma_start(rxg[:], rxrow[32 * g : 32 * (g + 1), :])
        b, c0 = j0 // s0, j0 % s0
        ib_ps = psum.tile([P, 4 * D], f32, tag=f"ib{tag}")
        for jj in range(4):
            j = j0 + jj
            nc.tensor.matmul(
                ib_ps[:, jj * D : (jj + 1) * D], ones_t[:],
                rxg[0:1, (j % 32) * P : (j % 32 + 1) * P],
                start=True, stop=True,
            )
        ib16 = pe_pool.tile([P, 4 * D], f16, tag=f"ib16{tag}")
        nc.scalar.activation(
            out=ib16[:], in_=ib_ps[:], func=mybir.ActivationFunctionType.Copy
        )
        res_ps = psum.tile([P, 4 * D], f32, tag=f"res{tag}")
        for jj in range(4):
            oh = pe_pool.tile([P, 4 * D], f16, tag=f"oh{tag}")
            for c in range(4):
                nc.vector.tensor_scalar(
                    out=oh[:, c * D : (c + 1) * D],
                    in0=ib16[:, jj * D : (jj + 1) * D],
                    scalar1=kvec[:, c : c + 1], scalar2=None,
                    op0=mybir.AluOpType.is_equal,
                )
            for c in range(4):
                nc.tensor.matmul(
                    res_ps[:, jj * D : (jj + 1) * D],
                    oh[:, c * D : (c + 1) * D],
                    cbf[:, c * D : (c + 1) * D],
                    start=(c == 0), stop=(c == 3),
                )
        nc.vector.tensor_copy(
            out=kv_tile[:, b, c0 : c0 + 4, :].rearrange("p c j -> p (c j)"),
            in_=res_ps[:],
        )


def _kv(nc, out, kv_tile, ctx_t, *, dho, row0, n_queues, call_counter):
    # kv_writeback: out[b, dhi, dho, 0:128] = kv_tile[dhi, b, dho, :], i.e.
    # the contiguous run at row b, offset row0 + (dhi*dho + c)*128.  One call
    # per batch: the interp materializes non-contiguous dst APs as packed
    # copies, so multi-batch calls (batch_stride != packed stride) misplace
    # batches > 0; a single-batch region is contiguous and always safe.
    out_v = (
        out.ap()[:, row0 : row0 + dho * P * D]
        .rearrange("b (dhi dho j) -> b dhi dho j", dhi=P, j=D)
    )
    for b in range(B_LOC):
        bs = slice(b, b + 1)
        nc.gpsimd.kv_writeback(
            out_v[bs],
            kv_tile[:, bs, :, :].transpose([0, 2, 1, 3]),
            ctx_t[:, bs],
            queue_num=call_counter[0] % n_queues,
        )
        call_counter[0] += 1


def _region(nc, pool, dram_pool, out, kv_tile, ctx_t, *, idx_dram, flip_dram,
            cb, dho, row0, tag, n_queues, call_counter, s0=0, emit_kv=True):
    """Full pipeline for one region (coarse or fine).

    Layouts (per batch b, point q = 512*j2 + 32*r + c for fine / analogous
    for coarse with strides (128, 8, 1)):
      compute tiles:  partition 16*b + r, free (j2, c)
      scratch SA:     [r, b, (c, j2)]  (contiguous 2*dho bytes * ... runs)
      W (wrapped):    [16*grp + r, 8*dho*b + 8*c + j2], grp-replicated
      gather dst:     kv_tile[:, b, c, :]
    """
    C = dho                    # c digit size
    FQ = J2 * C                # free size of compute tiles (per partition)
    WCOLS = B_LOC * J2 * C     # W columns
    f16 = mybir.dt.float16

    idx_t = pool.tile([P, FQ], i32, tag=f"idx{tag}")
    idx_flat = idx_dram.ap().rearrange("b h w -> b (h w)")
    u_flat = flip_dram.ap().rearrange("b h w k -> b (h w) k")
    u_t = pool.tile([P, FQ * BITS], f32, tag=f"u{tag}")
    for b in range(B_LOC):
        # idx on SP, u on ACT: HWDGE is shared but the two SEQs issue in
        # parallel, halving the serialized setup phase.
        nc.sync.dma_start(
            idx_t[16 * b : 16 * b + 16, :],
            idx_flat[b].rearrange("(j2 r c) -> r j2 c", j2=J2, r=R16),
        )
        nc.scalar.dma_start(
            u_t[16 * b : 16 * b + 16, :],
            u_flat[b].rearrange("(j2 r c) k -> r j2 (c k)", j2=J2, r=R16),
        )

    # flipmask: fm = sum_k (u_k < BER) * 2^k, accumulated plane by plane via
    # a small tmp tile (exact in f16: all values integers <= 511)
    u_v = u_t[:].rearrange("p (f k) -> p f k", k=BITS)
    fm_t = pool.tile([P, FQ], f16, tag=f"fm{tag}")
    tmp_t = pool.tile([P, FQ], f16, tag=f"tmp{tag}")
    with nc.allow_low_precision(reason="bit sums <= 511 are exact in f16"):
        for k in range(BITS):
            dst = fm_t if k == 0 else tmp_t
            nc.vector.tensor_scalar(
                out=dst[:],
                in0=u_v[:, :, k],
                scalar1=BER,
                scalar2=float(1 << k),
                op0=mybir.AluOpType.is_lt,
                op1=mybir.AluOpType.mult,
            )
            if k:
                nc.vector.tensor_tensor(
                    out=fm_t[:], in0=fm_t[:], in1=tmp_t[:],
                    op=mybir.AluOpType.add,
                )
    fm_i = pool.tile([P, FQ], i32, tag=f"fmi{tag}")
    nc.vector.tensor_copy(out=fm_i[:], in_=fm_t[:])
    rx_t = pool.tile([P, FQ], i32, tag=f"rx{tag}")
    nc.vector.tensor_tensor(
        out=rx_t[:], in0=idx_t[:], in1=fm_i[:], op=mybir.AluOpType.bitwise_xor
    )
    # (j2, c) -> (c, j2) free permute fused into the i32 -> i16 copy so the
    # scratch write below is contiguous on both sides.
    rx16 = pool.tile([P, FQ], i16, tag=f"rx16{tag}")
    nc.vector.tensor_copy(
        out=rx16[:].rearrange("p (c j2) -> p c j2", j2=J2),
        in_=rx_t[:].rearrange("p (j2 c) -> p c j2", j2=J2),
    )

    # DRAM round trip to the wrapped, group-replicated index tile W.
    # SA[r, b, (c, j2)]: partition 16b+r writes one contiguous 2*FQ-byte run.
    sa = dram_pool.tile([R16, B_LOC, FQ], i16, tag=f"sa{tag}")
    nc.scalar.dma_start(sa[:].transpose([1, 0, 2]), rx16[:])
    W = pool.tile([P, WCOLS], i16, tag=f"W{tag}")
    nc.scalar.dma_start(
        W[:],
        sa[:]
        .rearrange("r b f -> r (b f)")
        .unsqueeze(0)
        .broadcast_to([J2, R16, WCOLS]),
    )

    # One gather call per batch covering slots [s0, dho) (slots < s0 belong
    # to the PE path): position g = c*128 + p lands at kv_tile[p, b, s0+c, :],
    # reading W[g%16, b*8*dho + 8*s0 + g//16].
    for b in range(B_LOC):
        n_idx = (C - s0) * P
        nc.gpsimd.dma_gather(
            kv_tile[:, b, s0:C, :],
            cb.ap(),
            W[:, b * J2 * C + J2 * s0 : (b + 1) * J2 * C],
            n_idx,
            n_idx,
            D,
            queue_num=call_counter[0] % n_queues,
            single_packet=False,
        )
        call_counter[0] += 1

    if emit_kv:
        _kv(nc, out, kv_tile, ctx_t, dho=dho, row0=row0, n_queues=n_queues,
            call_counter=call_counter)


def build_nc():
    n_queues = int(os.environ.get("K_NQ", "4"))
    s0 = int(os.environ.get("K_S0", "12"))     # fine slots via PE per batch
    nc = bacc.Bacc(
        "TRN2", target_bir_lowering=False, debug=False, num_swdge_queues=n_queues,
        dynamic_dma_scratch_size=int(os.environ.get("K_RING", "53248")),
    )

    idx_c = nc.dram_tensor("idx_c", [B_LOC, HC, WC], i32, kind="ExternalInput")
    idx_f = nc.dram_tensor("idx_f", [B_LOC, HF, WF], i32, kind="ExternalInput")
    cb_c = nc.dram_tensor("codebook_c", [KC, D], f32, kind="ExternalInput")
    cb_f = nc.dram_tensor("codebook_f", [KF, D], f32, kind="ExternalInput")
    fu_c = nc.dram_tensor("flip_u_c", [B_LOC, HC, WC, BITS], f32, kind="ExternalInput")
    fu_f = nc.dram_tensor("flip_u_f", [B_LOC, HF, WF, BITS], f32, kind="ExternalInput")
    out = nc.dram_tensor("out", [B_LOC, OUT_ROW], f32, kind="ExternalOutput")

    with tile.TileContext(nc) as tc:
        with (
            tc.tile_pool(name="io", bufs=1) as pool,
            tc.tile_pool(name="pe", bufs=2) as pe_pool,
            tc.tile_pool(name="ps", bufs=2, space="PSUM") as psum,
            tc.tile_pool(name="dram", bufs=1, space="DRAM") as dram_pool,
        ):
            call_counter = [0]
            for _rep in range(int(os.environ.get("K_REPS", "1"))):
                ctx_t = pool.tile([P, B_LOC], i32, tag="ctx")
                nc.vector.memset(ctx_t[:], 0)
                if s0:
                    ident, ones_t, kvec = _pe_prep(nc, pool)
                    cbf16 = _cb16(nc, pool, cb_f, "f")
                kvf_t = pool.tile([P, B_LOC, DHOF, D], f32, tag="kvf")
                kvf = kvf_t[:]
                # The coarse region reuses the first DHOC slots of the fine
                # kv buffer (coarse writeback completes before fine gathers
                # overwrite it; subtile deps order the two).
                kvc = kvf[:, :, 0:DHOC, :]
                # Coarse first: its gathers feed the DMA engines while the
                # fine region's loads/bitflip prologue runs.
                _region(
                    nc, pool, dram_pool, out, kvc, ctx_t,
                    idx_dram=idx_c, flip_dram=fu_c, cb=cb_c, dho=DHOC,
                    row0=FINE_ROW, tag="c", n_queues=n_queues,
                    call_counter=call_counter,
                )
                _region(
                    nc, pool, dram_pool, out, kvf, ctx_t,
                    idx_dram=idx_f, flip_dram=fu_f, cb=cb_f, dho=DHOF,
                    row0=0, tag="f", n_queues=n_queues,
                    call_counter=call_counter, s0=s0, emit_kv=False,
                )
                if s0:
                    _pe_region(
                        nc, pool, pe_pool, psum, kvf,
                        idx_dram=idx_f, flip_dram=fu_f, cbf=cbf16, kvec=kvec,
                        ones_t=ones_t, ident=ident, dho=DHOF, s0=s0, tag="f",
                    )
                _kv(nc, out, kvf, ctx_t, dho=DHOF, row0=0, n_queues=n_queues,
                    call_counter=call_counter)

    nc.compile()
    return nc


_NC_CACHE = None


def _get_nc():
    global _NC_CACHE
    if _NC_CACHE is None:
        _NC_CACHE = build_nc()
    return _NC_CACHE


def _in_maps(idx_c, idx_f, codebook_c, codebook_f, flip_u_c, flip_u_f):
    maps = []
    for c in range(N_CORES):
        b0, b1 = c * B_LOC, (c + 1) * B_LOC
        maps.append(
            {
                "idx_c": np.ascontiguousarray(idx_c[b0:b1]),
                "idx_f": np.ascontiguousarray(idx_f[b0:b1]),
                "codebook_c": np.ascontiguousarray(codebook_c),
                "codebook_f": np.ascontiguousarray(codebook_f),
                "flip_u_c": np.ascontiguousarray(flip_u_c[b0:b1]),
                "flip_u_f": np.ascontiguousarray(flip_u_f[b0:b1]),
            }
        )
    return maps


class _AxonRunner:
    """Cached sharded PJRT executable for the axon path.

    run_bass_kernel_spmd rebuilds its jit closure (and retraces) on every
    call; caching the executable makes repeat kernel() calls cheap. Uses the
    same bass2jax machinery run_bass_kernel_spmd itself uses under axon.
    """

    def __init__(self, nc):
        import jax
        from jax.sharding import Mesh, NamedSharding, PartitionSpec
        from jax.experimental.shard_map import shard_map
        import concourse.bass2jax as b2j

        b2j.install_neuronx_cc_hook()
        self._jax = jax
        pname = nc.partition_id_tensor.name if nc.partition_id_tensor else None
        in_names, out_names, out_avals, zeros = [], [], [], []
        for alloc in nc.m.functions[0].allocations:
            if not isinstance(alloc, mybir.MemoryLocationSet):
                continue
            name = alloc.memorylocations[0].name
            if alloc.kind == "ExternalInput":
                if name != pname:
                    in_names.append(name)
            elif alloc.kind == "ExternalOutput":
                out_names.append(name)
                shape = tuple(alloc.tensor_shape)
                dtype = mybir.dt.np(alloc.dtype)
                out_avals.append(jax.core.ShapedArray(shape, dtype))
                zeros.append(np.zeros((N_CORES * shape[0], *shape[1:]), dtype))
        self.in_names = in_names
        all_in = in_names + out_names + ([pname] if pname else [])

        def _body(*args):
            ops = list(args)
            if pname is not None:
                ops.append(b2j.partition_id_tensor())
            return tuple(
                b2j._bass_exec_p.bind(
                    *ops,
                    out_avals=tuple(out_avals),
                    in_names=tuple(all_in),
                    out_names=tuple(out_names),
                    lowering_input_output_aliases=(),
                    sim_require_finite=True,
                    sim_require_nnan=True,
                    nc=nc,
                )
            )

        devices = jax.devices()[:N_CORES]
        mesh = Mesh(np.asarray(devices), ("core",))
        n = len(in_names) + len(out_names)
        self.sharded = jax.jit(
            shard_map(
                _body,
                mesh=mesh,
                in_specs=(PartitionSpec("core"),) * n,
                out_specs=(PartitionSpec("core"),) * len(out_names),
                check_rep=False,
            ),
            keep_unused=True,
        )
        self.sh = NamedSharding(mesh, PartitionSpec("core"))
        self.dev_zeros = [jax.device_put(z, self.sh) for z in zeros]

    def run(self, full):
        jax = self._jax
        dev_in = [jax.device_put(full[n], self.sh) for n in self.in_names]
        outs = self.sharded(*dev_in, *self.dev_zeros)
        return np.asarray(outs[0]).reshape(B, OUT_ROW)


_RUNNER = None


def kernel(idx_c, idx_f, codebook_c, codebook_f, flip_u_c, flip_u_f):
    from concourse._compat import axon_active

    if axon_active():
        global _RUNNER
        if _RUNNER is None:
            _RUNNER = _AxonRunner(_get_nc())
        full = {
            "idx_c": np.ascontiguousarray(idx_c),
            "idx_f": np.ascontiguousarray(idx_f),
            "codebook_c": np.tile(np.ascontiguousarray(codebook_c), (N_CORES, 1)),
            "codebook_f": np.tile(np.ascontiguousarray(codebook_f), (N_CORES, 1)),
            "flip_u_c": np.ascontiguousarray(flip_u_c),
            "flip_u_f": np.ascontiguousarray(flip_u_f),
        }
        return _RUNNER.run(full)

    nc = _get_nc()
    maps = _in_maps(idx_c, idx_f, codebook_c, codebook_f, flip_u_c, flip_u_f)
    res = run_bass_kernel_spmd(nc, maps, core_ids=list(range(N_CORES)))
    return np.concatenate([r["out"] for r in res.results], axis=0)



# revision 36
# speedup vs baseline: 1.2593x; 1.2593x over previous
"""Trainium2 Bass kernel for nn_AWGNIndexChannelWrapper.

Reference computation:
  rx_c = bitflip(idx_c, flip_u_c)  (9-bit symbols, per-bit XOR with (u < BER))
  rx_f = bitflip(idx_f, flip_u_f)
  out  = concat([codebook_f[rx_f].reshape(B, -1), codebook_c[rx_c].reshape(B, -1)], axis=1)

Key simplification: packing/unpacking 9-bit symbols with per-bit XOR is just
  rx = idx ^ flipmask,  flipmask = sum_k (u_k < BER) << k
and the clip is a no-op (9-bit values are already < 512).

Sharding: pure data parallel over the batch dim (64 batches -> 8 cores x 8).

v3 design (the kernel is HBM-wire-bound: ~21MB output write per core is the
floor, so every byte of DRAM gather read competes with the writeback):

- The wrapped index tile W (built once per region from the bitflipped rx via
  a DRAM round trip, exactly as v2) feeds BOTH gather mechanisms: position
  i = ((b*C + c)*8 + j2)*16 + r  maps to point q = C*(16*j2 + r) + c, i.e.
  slot (b, c) holds points q = C*p + c across partitions p -- the native
  dma_gather deposit layout AND, transposed, the ap_gather output order.

- Fine region: gathered ON-CHIP with gpsimd.ap_gather from a transposed
  codebook cbT[d=partition, 512] f32 resident in SBUF (zero HBM traffic).
  The transposed result [d, points] is converted f32->f16 (DVE), PE-transposed
  per 128-point slot into PSUM (f16 round trip, rel err ~2^-11 << 2e-2), and
  copied (DVE/ACT alternating) into the kv tile [dhi, b, dho, j], then written
  out with one kv_writeback per batch (SWDGE 16-partition-stripe descriptors).
  K_A slots per batch go through ap_gather (default all 32); the rest use the
  f16 dma_gather below.

- Coarse region: ONE multi-batch f16 dma_gather (256B descriptors from a
  f16 DRAM codebook copy -- half the read traffic of f32) into a staging
  tile, converted f32<-f16 into the kv tile's slots [24,32) (reused as the
  coarse region buffer), and written out with ONE plain HWDGE DMA whose
  per-partition runs are contiguous 4KB (batch point count 1024 = 128*8).

- Engine budget per core (model): Pool ~60us (ap_gather + SWDGE desc gen),
  DVE ~48us (bitflip, converts, PSUM copies), ACT ~45us (PSUM copies),
  PE ~33us (transposes), DMA wire ~72us (output 21MB + inputs + coarse).
"""

import os

import numpy as np

import concourse.bacc as bacc
import concourse.mybir as mybir
import concourse.tile as tile
from concourse.bass_utils import run_bass_kernel_spmd

# Problem constants (hardcoded per harness contract).
BER = 0.02
BITS = 9
KC = KF = 512
B, HC, WC, HF, WF, D = 64, 32, 32, 64, 64, 128

N_CORES = 8
B_LOC = B // N_CORES          # 8 batches per core
P = 128
QF = HF * WF                  # 4096 fine points per batch
QC = HC * WC                  # 1024 coarse points per batch
DHOF = QF // P                # 32 fine slots per batch (d_head_outer)
DHOC = QC // P                # 8 coarse slots per batch
FINE_ROW = QF * D             # 524288 f32 per output row (fine region)
COARSE_ROW = QC * D           # 131072 f32 per output row (coarse region)
OUT_ROW = FINE_ROW + COARSE_ROW
C_OVL = DHOF - DHOC           # coarse overlays fine kv slots [24, 32)

f32 = mybir.dt.float32
f16 = mybir.dt.float16
i32 = mybir.dt.int32
i16 = mybir.dt.int16

J2 = 8                        # partition-group digit (p = 16*j2 + r)
R16 = 16                      # wrapped-row digit


FQ_MAX = J2 * DHOF


def _bf_loads(nc, pool, *, idx_dram, flip_dram, dho, tag, idx_eng=None,
              u_eng=None, u_first=False):
    """Dep-free input loads at layout [partition 16b+r, free (j2, c)]
    (point q = dho*(16*j2 + r) + c).  The wrapped layout pins r (=
    partition % 16) to a middle digit of q, so both loads have irreducibly
    4D in-APs and must split per batch.  With u_first, all u DMAs issue
    before any idx DMA (the bitflip compare chain needs u only)."""
    C = dho
    FQ = J2 * C
    idx_eng = idx_eng or nc.sync
    u_eng = u_eng or nc.scalar
    idx_t = pool.tile([P, FQ], i32, tag=f"idx{tag}")
    u_t = pool.tile([P, FQ * BITS], f32, tag=f"u{tag}")
    idx_flat = idx_dram.ap().rearrange("b h w -> b (h w)")
    u_flat = flip_dram.ap().rearrange("b h w k -> b (h w) k")

    def load_u(b):
        u_eng.dma_start(
            u_t[16 * b : 16 * b + 16, :],
            u_flat[b].rearrange("(j2 r c) k -> r j2 (c k)", j2=J2, r=R16),
        )

    def load_idx(b):
        idx_eng.dma_start(
            idx_t[16 * b : 16 * b + 16, :],
            idx_flat[b].rearrange("(j2 r c) -> r j2 c", j2=J2, r=R16),
        )

    if u_first:
        for b in range(B_LOC):
            load_u(b)
        for b in range(B_LOC):
            load_idx(b)
    else:
        for b in range(B_LOC):
            load_idx(b)
            load_u(b)
    return idx_t, u_t


def _bf_compute(nc, pool, idx_t, u_t, dho):
    """rx = idx ^ flipmask on DVE; returns rx16 with free dims permuted to
    (c, j2).  Chain scratch is shared between regions (coarse runs strictly
    before fine on DVE)."""
    C = dho
    FQ = J2 * C
    u_v = u_t[:].rearrange("p (f k) -> p f k", k=BITS)
    fm_t = pool.tile([P, FQ_MAX], f16, tag="bf_fm")
    tmp_t = pool.tile([P, FQ_MAX], f16, tag="bf_tmp")
    with nc.allow_low_precision(reason="bit sums <= 511 are exact in f16"):
        for k in range(BITS):
            dst = fm_t if k == 0 else tmp_t
            nc.vector.tensor_scalar(
                out=dst[:, 0:FQ],
                in0=u_v[:, :, k],
                scalar1=BER,
                scalar2=float(1 << k),
                op0=mybir.AluOpType.is_lt,
                op1=mybir.AluOpType.mult,
            )
            if k:
                nc.vector.tensor_tensor(
                    out=fm_t[:, 0:FQ], in0=fm_t[:, 0:FQ], in1=tmp_t[:, 0:FQ],
                    op=mybir.AluOpType.add,
                )
    fm_i = pool.tile([P, FQ_MAX], i32, tag="bf_fmi")
    nc.vector.tensor_copy(out=fm_i[:, 0:FQ], in_=fm_t[:, 0:FQ])
    rx_t = pool.tile([P, FQ_MAX], i32, tag="bf_rx")
    nc.vector.tensor_tensor(
        out=rx_t[:, 0:FQ], in0=idx_t[:], in1=fm_i[:, 0:FQ],
        op=mybir.AluOpType.bitwise_xor,
    )
    rx16 = pool.tile([P, FQ_MAX], i16, tag="bf_rx16")
    nc.vector.tensor_copy(
        out=rx16[:, 0:FQ].rearrange("p (c j2) -> p c j2", j2=J2),
        in_=rx_t[:, 0:FQ].rearrange("p (j2 c) -> p c j2", j2=J2),
    )
    return rx16


def _build_W(nc, pool, dram_pool, rx16, *, dho, tag, split=None):
    """DRAM round trip to the wrapped, group-replicated index tile W.

    SA[r, b, (c, j2)]: partition 16b+r writes one contiguous run; the read
    back broadcasts each 16-row stripe to all 8 partition groups.  W column
    ((b*C + c)*8 + j2) holds rx[b, q = C*(16*j2+r) + c].

    With split=A (< dho), W is reordered into two col regions so multi-call
    gathers get contiguous index slices: [(b, c < A) cols][(b, c >= A) cols].
    """
    C = dho
    FQ = J2 * C
    WCOLS = B_LOC * FQ
    sa = dram_pool.tile([R16, B_LOC, FQ], i16, tag=f"sa{tag}")
    nc.sync.dma_start(sa[:].transpose([1, 0, 2]), rx16[:, 0:FQ])
    W = pool.tile([P, WCOLS], i16, tag=f"W{tag}")
    if split is None or split == C:
        nc.sync.dma_start(
            W[:],
            sa[:]
            .rearrange("r b f -> r (b f)")
            .unsqueeze(0)
            .broadcast_to([J2, R16, WCOLS]),
        )
    else:
        A = split
        n1 = B_LOC * A * J2
        nc.sync.dma_start(
            W[:, 0:n1].rearrange("p (b f) -> p b f", b=B_LOC),
            sa[:, :, 0 : A * J2]
            .unsqueeze(0)
            .broadcast_to([J2, R16, B_LOC, A * J2]),
        )
        nc.sync.dma_start(
            W[:, n1:WCOLS].rearrange("p (b f) -> p b f", b=B_LOC),
            sa[:, :, A * J2 : FQ]
            .unsqueeze(0)
            .broadcast_to([J2, R16, B_LOC, FQ - A * J2]),
        )
    return W


def _make_cb16_dram(nc, pool, dram_pool, cb, tag):
    """f16 copy of a [512, 128] f32 codebook in DRAM (256B gather rows)."""
    st = pool.tile([P, 4, D], f32, tag=f"cbst{tag}")
    nc.sync.dma_start(
        st[:], cb.ap().rearrange("(c p) j -> p c j", p=P)
    )
    st16 = pool.tile([P, 4, D], f16, tag=f"cbst16{tag}")
    with nc.allow_low_precision(reason="f16 codebook, rel err 2^-11"):
        nc.vector.tensor_copy(out=st16[:], in_=st[:])
    cb16 = dram_pool.tile([KC, D], f16, tag=f"cb16{tag}")
    nc.sync.dma_start(
        cb16[:].rearrange("(c p) j -> p c j", p=P), st16[:]
    )
    return cb16


def _make_cbT(nc, pool, psum, cb, identf, tag):
    """Transposed codebook cbT[d=partition, 512 entries] f32 in SBUF via 4
    exact PE f32 transposes (x*1.0 + 0 is exact, also under bf16x3)."""
    st = pool.tile([P, 4, D], f32, tag=f"cbt_st{tag}")
    nc.sync.dma_start(
        st[:], cb.ap().rearrange("(c p) j -> p c j", p=P)
    )
    cbT = pool.tile([P, 4 * P, 1], f32, tag=f"cbT{tag}")
    tp = psum.tile([P, 4, P], f32, tag=f"cbt_ps{tag}")
    for c in range(4):
        nc.tensor.transpose(tp[:, c, :], st[:, c, :], identf[:])
    nc.vector.tensor_copy(
        out=cbT[:].rearrange("p e one -> p (e one)"),
        in_=tp[:].rearrange("p c e -> p (c e)"),
    )
    return cbT


def _kv_batch(nc, out, kv_b, ctx_t, b, *, dho, row0, n_queues, call_counter):
    # kv_writeback: out[b, dhi, dho, 0:128] = kv_b[dhi, 0, dho, :]. One call
    # per batch: multi-batch calls (batch_stride != packed stride) misplace
    # batches > 0 in this stack's descriptor lowering.  kv_b is a rotating
    # per-batch buffer [P, 1, dho, D].
    out_v = (
        out.ap()[:, row0 : row0 + dho * P * D]
        .rearrange("b (dhi dho j) -> b dhi dho j", dhi=P, j=D)
    )
    nc.gpsimd.kv_writeback(
        out_v[b : b + 1],
        kv_b[:, :, :, :].transpose([0, 2, 1, 3]),
        ctx_t[:, b : b + 1],
        queue_num=call_counter[0] % n_queues,
    )
    call_counter[0] += 1


def build_nc():
    n_queues = int(os.environ.get("K_NQ", "4"))
    A = int(os.environ.get("K_A", "32"))       # fine slots per batch via ap_gather
    assert 0 <= A <= DHOF and A % 4 == 0
    nc = bacc.Bacc(
        "TRN2", target_bir_lowering=False, debug=False,
        num_swdge_queues=n_queues,
        dynamic_dma_scratch_size=int(os.environ.get("K_RING", "53248")),
    )

    idx_c = nc.dram_tensor("idx_c", [B_LOC, HC, WC], i32, kind="ExternalInput")
    idx_f = nc.dram_tensor("idx_f", [B_LOC, HF, WF], i32, kind="ExternalInput")
    cb_c = nc.dram_tensor("codebook_c", [KC, D], f32, kind="ExternalInput")
    cb_f = nc.dram_tensor("codebook_f", [KF, D], f32, kind="ExternalInput")
    fu_c = nc.dram_tensor("flip_u_c", [B_LOC, HC, WC, BITS], f32, kind="ExternalInput")
    fu_f = nc.dram_tensor("flip_u_f", [B_LOC, HF, WF, BITS], f32, kind="ExternalInput")
    out = nc.dram_tensor("out", [B_LOC, OUT_ROW], f32, kind="ExternalOutput")

    with tile.TileContext(nc) as tc:
        with (
            tc.tile_pool(name="io", bufs=1) as pool,
            tc.tile_pool(name="gt", bufs=2) as gt_pool,
            tc.tile_pool(name="kv", bufs=4) as kv_pool,
            tc.tile_pool(name="ps", bufs=7, space="PSUM") as psum,
            tc.tile_pool(name="ps1", bufs=1, space="PSUM") as psum1,
            tc.tile_pool(name="dram", bufs=1, space="DRAM") as dram_pool,
        ):
            call_counter = [0]
            for _rep in range(int(os.environ.get("K_REPS", "1"))):
                ctx_t = pool.tile([P, B_LOC], i32, tag="ctx")
                nc.vector.memset(ctx_t[:], 0)
                from concourse.masks import make_identity
                ident16 = pool.tile([P, P], f16, tag="ident16")
                make_identity(nc, ident16[:])
                identf = pool.tile([P, P], f32, tag="identf")
                make_identity(nc, identf[:])

                # ---- phase 0+1: the FINE critical path owns the head:
                # u_f/idx_f loads -> bitflip -> sa_f -> W_f -> first gather.
                # Everything coarse is deferred past W_f so its DMAs never
                # occupy the shared HWDGE device before W_f is built.
                cbT_f = _make_cbT(nc, pool, psum1, cb_f, identf, "f")
                # ALL input loads on the ACT HWDGE queue, in critical-path
                # order (u_f first -- the bitflip compare chain needs it
                # before idx): the shared HWDGE device grants in issue order,
                # so priority = program order on one SEQ.
                idx_tf, u_tf = _bf_loads(
                    nc, pool, idx_dram=idx_f, flip_dram=fu_f, dho=DHOF,
                    tag="f", idx_eng=nc.scalar, u_eng=nc.scalar, u_first=True,
                )
                cb16_f = None
                if A < DHOF:
                    cb16_f = _make_cb16_dram(nc, pool, dram_pool, cb_f, "f")
                rx16_f = _bf_compute(nc, pool, idx_tf, u_tf, DHOF)
                W_f = _build_W(
                    nc, pool, dram_pool, rx16_f, dho=DHOF, tag="f", split=A
                )

                # coarse region prep (its gathers interleave after batch 0).
                # Coarse goes straight to f32 dma_gather into kvc -- no f16
                # staging/converts: its 2.1MB of extra read wire is cheaper
                # than the engine time and serialization the converts cost.
                # Both coarse loads ride the ACT HWDGE queue so they never
                # delay the SP-issued sa_f/W_f on the shared HWDGE device.
                idx_tc, u_tc = _bf_loads(
                    nc, pool, idx_dram=idx_c, flip_dram=fu_c, dho=DHOC,
                    tag="c", idx_eng=nc.scalar, u_eng=nc.scalar,
                )
                rx16_c = _bf_compute(nc, pool, idx_tc, u_tc, DHOC)
                W_c = _build_W(nc, pool, dram_pool, rx16_c, dho=DHOC, tag="c")

                # coarse result tile (independent of the fine kv buffers)
                kvc = pool.tile([P, B_LOC, DHOC, D], f32, tag="kvc")
                BH = B_LOC // 2

                def coarse_gather_half(half):
                    nc.gpsimd.dma_gather(
                        kvc[:, BH * half : BH * (half + 1), :, :].rearrange(
                            "p b c j -> p (b c) j"
                        ),
                        cb_c.ap(),
                        W_c[:, half * BH * DHOC * J2 : (half + 1) * BH * DHOC * J2],
                        BH * QC, BH * QC, D,
                        queue_num=0, single_packet=False,
                    )

                def coarse_out():
                    # ONE HWDGE DMA: per-partition runs are contiguous 4KB
                    # (coarse point q = 8p + c lives at FINE_ROW + q*128)
                    nc.sync.dma_start(
                        out.ap()[:, FINE_ROW:]
                        .rearrange("b (dhi c j) -> dhi b (c j)", dhi=P, c=DHOC),
                        kvc[:].rearrange("p b c j -> p b (c j)"),
                    )

                # ---- phase 2: per fine batch: ap_gather chunks -> f16
                # convert -> PE transpose -> PSUM copy into a rotating
                # per-batch kv buffer -> kv_writeback.  Slots [A, 32) come
                # from a per-batch f16 dma_gather instead.  Coarse gathers /
                # converts / out are interleaved after batches 0/1/2.
                HALF = 8 if A >= 8 else A
                nf16 = DHOF - A
                copy_rr = [0]
                n1 = B_LOC * A * J2
                for b in range(B_LOC):
                    kvb_t = kv_pool.tile([P, 1, DHOF, D], f32, tag="kvb")
                    kvb = kvb_t[:]
                    if nf16:
                        kvf16 = gt_pool.tile([P, nf16, D], f16, tag="kvf16")
                        nc.gpsimd.dma_gather(
                            kvf16[:], cb16_f[:],
                            W_f[:, n1 + b * nf16 * J2 : n1 + (b + 1) * nf16 * J2],
                            nf16 * P, nf16 * P, D,
                            queue_num=1 + (b % max(1, n_queues - 2)),
                            single_packet=False,
                        )
                        dst = kvb[:, 0, A:DHOF, :].rearrange("p c j -> p (c j)")
                        src = kvf16[:].rearrange("p c j -> p (c j)")
                        with nc.allow_low_precision(reason="f16, 2^-11"):
                            if b % 2 == 0:
                                nc.vector.tensor_copy(out=dst, in_=src)
                            else:
                                nc.scalar.activation(
                                    out=dst, in_=src,
                                    func=mybir.ActivationFunctionType.Copy,
                                )
                    for h0 in range(0, A, HALF or 1):
                        if A == 0:
                            break
                        hn = min(HALF, A - h0)
                        ncols = hn * P
                        gT = gt_pool.tile([P, HALF * P, 1], f32, tag="gT")
                        nc.gpsimd.ap_gather(
                            gT[:, 0:ncols, :], cbT_f[:],
                            W_f[:, (b * A + h0) * J2 : (b * A + h0 + hn) * J2],
                            P, KF, 1, ncols,
                        )
                        gT16 = gt_pool.tile([P, HALF * P], f16, tag="gT16")
                        with nc.allow_low_precision(reason="f16, 2^-11"):
                            nc.vector.tensor_copy(
                                out=gT16[:, 0:ncols],
                                in_=gT[:, 0:ncols, :].rearrange(
                                    "p a one -> p (a one)"
                                ),
                            )
                        for g0 in range(0, hn, 4):
                            tp = psum.tile([P, 4, P], f16, tag="tp")
                            for k in range(4):
                                cc = g0 + k
                                nc.tensor.transpose(
                                    tp[:, k, :],
                                    gT16[:, cc * P : (cc + 1) * P],
                                    ident16[:],
                                )
                            c0 = h0 + g0
                            dst = kvb[:, 0, c0 : c0 + 4, :].rearrange(
                                "p c j -> p (c j)"
                            )
                            src = tp[:].rearrange("p k e -> p (k e)")
                            with nc.allow_low_precision(reason="f16, 2^-11"):
                                # PSUM copies: first 5 groups of each batch
                                # on ACT, last 3 on the faster DVE so the
                                # batch's kv_writeback isn't tail-gated by
                                # the slower ACT queue (ACT:DVE stays 5:3)
                                if copy_rr[0] % 8 < 5:
                                    nc.scalar.activation(
                                        out=dst, in_=src,
                                        func=mybir.ActivationFunctionType.Copy,
                                    )
                                else:
                                    nc.vector.tensor_copy(out=dst, in_=src)
                            copy_rr[0] += 1
                    _kv_batch(
                        nc, out, kvb, ctx_t, b,
                        dho=DHOF, row0=0, n_queues=n_queues,
                        call_counter=call_counter,
                    )
                    # interleave the coarse region into the fine pipeline
                    if b == 0:
                        coarse_gather_half(0)
                        coarse_gather_half(1)
                    elif b == 1:
                        coarse_out()

    nc.compile()
    return nc


_NC_CACHE = None


def _get_nc():
    global _NC_CACHE
    if _NC_CACHE is None:
        _NC_CACHE = build_nc()
    return _NC_CACHE


def _in_maps(idx_c, idx_f, codebook_c, codebook_f, flip_u_c, flip_u_f):
    maps = []
    for c in range(N_CORES):
        b0, b1 = c * B_LOC, (c + 1) * B_LOC
        maps.append(
            {
                "idx_c": np.ascontiguousarray(idx_c[b0:b1]),
                "idx_f": np.ascontiguousarray(idx_f[b0:b1]),
                "codebook_c": np.ascontiguousarray(codebook_c),
                "codebook_f": np.ascontiguousarray(codebook_f),
                "flip_u_c": np.ascontiguousarray(flip_u_c[b0:b1]),
                "flip_u_f": np.ascontiguousarray(flip_u_f[b0:b1]),
            }
        )
    return maps


class _AxonRunner:
    """Cached sharded PJRT executable for the axon path.

    run_bass_kernel_spmd rebuilds its jit closure (and retraces) on every
    call; caching the executable makes repeat kernel() calls cheap. Uses the
    same bass2jax machinery run_bass_kernel_spmd itself uses under axon.
    """

    def __init__(self, nc):
        import jax
        from jax.sharding import Mesh, NamedSharding, PartitionSpec
        from jax.experimental.shard_map import shard_map
        import concourse.bass2jax as b2j

        b2j.install_neuronx_cc_hook()
        self._jax = jax
        pname = nc.partition_id_tensor.name if nc.partition_id_tensor else None
        in_names, out_names, out_avals, zeros = [], [], [], []
        for alloc in nc.m.functions[0].allocations:
            if not isinstance(alloc, mybir.MemoryLocationSet):
                continue
            name = alloc.memorylocations[0].name
            if alloc.kind == "ExternalInput":
                if name != pname:
                    in_names.append(name)
            elif alloc.kind == "ExternalOutput":
                out_names.append(name)
                shape = tuple(alloc.tensor_shape)
                dtype = mybir.dt.np(alloc.dtype)
                out_avals.append(jax.core.ShapedArray(shape, dtype))
                zeros.append(np.zeros((N_CORES * shape[0], *shape[1:]), dtype))
        self.in_names = in_names
        all_in = in_names + out_names + ([pname] if pname else [])

        def _body(*args):
            ops = list(args)
            if pname is not None:
                ops.append(b2j.partition_id_tensor())
            return tuple(
                b2j._bass_exec_p.bind(
                    *ops,
                    out_avals=tuple(out_avals),
                    in_names=tuple(all_in),
                    out_names=tuple(out_names),
                    lowering_input_output_aliases=(),
                    sim_require_finite=True,
                    sim_require_nnan=True,
                    nc=nc,
                )
            )

        devices = jax.devices()[:N_CORES]
        mesh = Mesh(np.asarray(devices), ("core",))
        n = len(in_names) + len(out_names)
        self.sharded = jax.jit(
            shard_map(
                _body,
                mesh=mesh,
                in_specs=(PartitionSpec("core"),) * n,
                out_specs=(PartitionSpec("core"),) * len(out_names),
                check_rep=False,
            ),
            keep_unused=True,
        )
        self.sh = NamedSharding(mesh, PartitionSpec("core"))
        self.dev_zeros = [jax.device_put(z, self.sh) for z in zeros]

    def run(self, full):
        jax = self._jax
        dev_in = [jax.device_put(full[n], self.sh) for n in self.in_names]
        outs = self.sharded(*dev_in, *self.dev_zeros)
        return np.asarray(outs[0]).reshape(B, OUT_ROW)


_RUNNER = None


def kernel(idx_c, idx_f, codebook_c, codebook_f, flip_u_c, flip_u_f):
    from concourse._compat import axon_active

    if axon_active():
        global _RUNNER
        if _RUNNER is None:
            _RUNNER = _AxonRunner(_get_nc())
        full = {
            "idx_c": np.ascontiguousarray(idx_c),
            "idx_f": np.ascontiguousarray(idx_f),
            "codebook_c": np.tile(np.ascontiguousarray(codebook_c), (N_CORES, 1)),
            "codebook_f": np.tile(np.ascontiguousarray(codebook_f), (N_CORES, 1)),
            "flip_u_c": np.ascontiguousarray(flip_u_c),
            "flip_u_f": np.ascontiguousarray(flip_u_f),
        }
        return _RUNNER.run(full)

    nc = _get_nc()
    maps = _in_maps(idx_c, idx_f, codebook_c, codebook_f, flip_u_c, flip_u_f)
    res = run_bass_kernel_spmd(nc, maps, core_ids=list(range(N_CORES)))
    return np.concatenate([r["out"] for r in res.results], axis=0)
